# revision 17
# baseline (speedup 1.0000x reference)
"""Trainium2 Bass kernel for nn_LongDistanceAttention (GNN message passing).

Strategy (8 NeuronCores, SPMD, node/row sharding):
  Each core owns a 512-row block of nodes. All N x N score/attention work is
  done on the transposed layout [j(source, partitions), i(local rows, free)]:
    - stage-1 GAT: e.T[j,i] = lrelu(s_i[i] + s_j[j]) built on DVE;
      E = exp(e.T) * A.T-block; (E @ [Wh | 1 | 0]).T accumulated on PE gives
      numerator and row-sum at once; h_local = gelu(U / Z). Softmax without
      max-subtraction (validated |e|<6, |scores|<15).
    - h (natural, bf16, with ones column) and Wa.T blocks (f32) are
      all-gathered.
    - k-hop masks: A^k reachability via fp8 DoubleRow matmuls (exact: inputs
      are 0/1, accumulation in fp32 PSUM), binarized after each hop.
      Transposed recurrence: M_k = A.T @ M_{k-1} with lhsT = A8 (full fp8 A,
      all-gathered, streamed), rhs = previous binary mask column-block.
    - per hop: E_k = exp(scores.T) * mask_k (bf16*fp8 mixed on DVE),
      U.T = (h_aug.T)(E_k) with ones column giving Z; out.T += U.T * (1/Z).
  Final: Y.T = W_out.T @ out.T + b_out, output per core [128, 512] = block.T.

DMA queues: SP (nc.sync) carries the A-block load + fp8-A mask stream;
ACT (nc.scalar) carries everything else, so the two pipelines don't
head-of-line block each other.
"""

import os
import sys

import numpy as np

sys.path.insert(0, "/opt/trn_rl_repo")

import concourse.bass as bass  # noqa: E402
import concourse.mybir as mybir  # noqa: E402
import concourse.tile as tile  # noqa: E402
from concourse import bacc  # noqa: E402
from concourse.bass_utils import run_bass_kernel_spmd  # noqa: E402
from concourse.masks import make_identity  # noqa: E402

P = 128
N = 4096
NB = N // P            # 32 j-chunks
HID = 256
OUT_DIM = 128
NCORES = 8
LOC = N // NCORES      # 512 local rows per core
LB = LOC // P          # 4 local partition chunks
ALPHA = 0.2

F32 = mybir.dt.float32
F32R = mybir.dt.float32r
BF16 = mybir.dt.bfloat16
FP8 = mybir.dt.float8e4

MASK_MODE = os.environ.get("MASK_MODE", "fp8dr")

_CACHE = {}
last_in_maps = None


def build_kernel():
    nc = bacc.Bacc(
        "TRN2",
        target_bir_lowering=False,
        debug=False,
        enable_asserts=False,
        num_devices=NCORES,
    )

    # ---- kernel I/O ----
    X_d = nc.dram_tensor("X", [N, HID], F32, kind="ExternalInput")
    Xloc_d = nc.dram_tensor("X_loc", [LOC, HID], F32, kind="ExternalInput")
    Ablk_d = nc.dram_tensor("A_blk", [LOC, N], F32, kind="ExternalInput")
    Ws_d = nc.dram_tensor("W_s", [HID, HID], F32, kind="ExternalInput")
    r_d = nc.dram_tensor("r", [2 * HID, 1], F32, kind="ExternalInput")
    Wl_d = nc.dram_tensor("W_l", [HID, HID], F32, kind="ExternalInput")
    Wo_d = nc.dram_tensor("W_out", [HID, OUT_DIM], F32, kind="ExternalInput")
    bo_d = nc.dram_tensor("b_out", [OUT_DIM], F32, kind="ExternalInput")
    out_d = nc.dram_tensor("out", [OUT_DIM, LOC], F32, kind="ExternalOutput")

    # ---- internal DRAM ----
    a8_loc = nc.dram_tensor("a8_loc", [LOC, N], FP8)
    a8_all = nc.dram_tensor("a8_all", [N, N], FP8, addr_space="Shared")
    haug_loc = nc.dram_tensor("haug_loc", [LOC, HID + 2], BF16)
    haug_all = nc.dram_tensor("haug_all", [N, HID + 2], BF16, addr_space="Shared")
    wat_loc = nc.dram_tensor("wat_loc", [HID, LOC], F32)
    wat_all = nc.dram_tensor("wat_all", [HID * NCORES, LOC], F32,
                             addr_space="Shared")

    groups = [list(range(NCORES))]

    with tile.TileContext(nc) as tc:
        with (
            tc.tile_pool(name="const", bufs=1) as cpool,
            tc.tile_pool(name="small", bufs=1) as sm,
            tc.tile_pool(name="maskp", bufs=1) as mp,
            tc.tile_pool(name="wk", bufs=1) as wk,
            tc.tile_pool(name="pp", bufs=1, space="PSUM") as pp,
        ):
            # =========== constants / weights (ACT queue) ===========
            ident = cpool.tile([P, P], F32)
            make_identity(nc, ident)
            ident_r = cpool.tile([P, P], F32R)
            nc.vector.tensor_copy(ident_r[:], ident[:])
            Ws_sb = cpool.tile([P, 2, HID], F32R)
            nc.scalar.dma_start(
                Ws_sb[:], Ws_d.ap().rearrange("(k p) m -> p k m", p=P).bitcast(F32R)
            )
            Wl_sb = cpool.tile([P, 2, HID], F32R)
            nc.scalar.dma_start(
                Wl_sb[:], Wl_d.ap().rearrange("(k p) m -> p k m", p=P).bitcast(F32R)
            )
            Wo_sb = cpool.tile([P, 2, OUT_DIM], F32R)
            nc.scalar.dma_start(
                Wo_sb[:], Wo_d.ap().rearrange("(k p) m -> p k m", p=P).bitcast(F32R)
            )
            r_sb = cpool.tile([P, 4], F32R)
            nc.scalar.dma_start(
                r_sb[:], r_d.ap().rearrange("(c p) o -> p (c o)", p=P).bitcast(F32R)
            )
            bo_sb = cpool.tile([P, 1], F32)
            nc.scalar.dma_start(bo_sb[:], bo_d.ap().rearrange("(o p) -> p o", p=P))
            # W_s transposed (for s = X @ (W_s @ [r1 r2]))
            WsT = cpool.tile([P, 2, HID], F32R)
            for k2 in range(2):
                for f2 in range(2):
                    pws = pp.tile([P, P], F32R, tag="st", bufs=2, name="pws")
                    nc.tensor.transpose(
                        pws[:], Ws_sb[:, f2, k2 * P : (k2 + 1) * P], ident_r[:]
                    )
                    nc.vector.tensor_copy(
                        WsT[:, k2, f2 * P : (f2 + 1) * P], pws[:]
                    )
            rp = r_sb.rearrange("p (h c) -> p c h", c=2)
            w12 = cpool.tile([P, 2, 2], F32R)
            for mc in range(2):
                pw12 = pp.tile([P, 2], F32, tag="bcast", bufs=1, name="pw12")
                for kc in range(2):
                    nc.tensor.matmul(
                        pw12[:],
                        WsT[:, kc, mc * P : (mc + 1) * P],
                        rp[:, kc, :],
                        start=(kc == 0),
                        stop=(kc == 1),
                    )
                nc.vector.tensor_copy(w12[:, mc], pw12[:])

            # mask tiles (persist across hops)
            M0 = mp.tile([P, NB, LOC], FP8, name="M0")
            M1 = mp.tile([P, NB, LOC], FP8, name="M1")
            M2 = mp.tile([P, NB, LOC], FP8, name="M2")

            # small persistent tiles
            hT = sm.tile([P, 2, LOC], F32R, name="hT")
            hnat = sm.tile([P, LB, HID + 2], BF16, name="hnat")
            outT = sm.tile([P, 2, LOC], F32R, name="outT")
            WaTloc = sm.tile([P, 2, LOC], F32R, name="WaTloc")

            # =========== phase 1: A-block prep (loads on SP, stores ACT) =====
            NQ = 2048
            with tc.tile_pool(name="atp", bufs=1) as atp:
                At_bf = atp.tile([P, NB, LOC], BF16, name="At_bf")
                with tc.tile_pool(name="aprep", bufs=1) as aprep:
                    # two groups of 4 chunks: all loads/casts/stores first so
                    # the a8_loc writes (and thus the all-gather) are not
                    # paced by the transpose-copy DVE work
                    chunks = [(ic, nh) for ic in range(LB) for nh in range(2)]
                    for g in range(2):
                        group = chunks[4 * g : 4 * g + 4]
                        tiles = []
                        for ic, nh in group:
                            sl_r = slice(ic * P, (ic + 1) * P)
                            sl_c = slice(nh * NQ, (nh + 1) * NQ)
                            ablk = aprep.tile(
                                [P, NQ], F32R, tag="ablk", bufs=4, name="ablk"
                            )
                            nc.sync.dma_start(
                                ablk[:], Ablk_d.ap()[sl_r, sl_c].bitcast(F32R)
                            )
                            a8q = aprep.tile(
                                [P, NQ], FP8, tag="a8q", bufs=4, name="a8q"
                            )
                            nc.vector.tensor_copy(a8q[:], ablk[:])
                            nc.scalar.dma_start(a8_loc.ap()[sl_r, sl_c], a8q[:])
                            tiles.append(ablk)
                        for (ic, nh), ablk in zip(group, tiles):
                            for jq in range(NQ // P):
                                jc = nh * (NQ // P) + jq
                                ptA = pp.tile([P, P], F32R, tag="mask", bufs=2,
                                              name="ptA")
                                nc.tensor.transpose(
                                    ptA[:], ablk[:, jq * P : (jq + 1) * P],
                                    ident_r[:],
                                )
                                nc.vector.tensor_copy(
                                    At_bf[:, jc, ic * P : (ic + 1) * P], ptA[:]
                                )
                                nc.vector.tensor_copy(
                                    M0[:, jc, ic * P : (ic + 1) * P], ptA[:]
                                )
                # A8 all-gather: trigger as early as possible (only depends
                # on the a8_loc stores)
                nc.gpsimd.collective_compute(
                    "AllGather",
                    mybir.AluOpType.bypass,
                    ins=[a8_loc[:, :]],
                    outs=[a8_all[:, :]],
                    replica_groups=groups,
                )

                # =========== phase 2: Wh_aug, s vectors (X on ACT queue) =====
                with tc.tile_pool(name="s1pool", bufs=1) as s1pool:
                    Wh_aug = s1pool.tile([P, NB, HID + 2], F32R)
                    onez = s1pool.tile([P, NB, 2], F32)
                    nc.vector.memset(onez[:, :, 0:1], 1.0)
                    nc.vector.memset(onez[:, :, 1:2], 0.0)
                    nc.vector.tensor_copy(Wh_aug[:, :, HID : HID + 2], onez[:])
                    s_nat = s1pool.tile([P, NB], F32)

                    for o in range(NB):
                        xchunk = wk.tile([P, HID], F32R, tag="xw", bufs=6)
                        nc.scalar.dma_start(
                            xchunk[:],
                            X_d.ap()[o * P : (o + 1) * P, :].bitcast(F32R),
                        )
                        xt = wk.tile([P, 2, P], F32R, tag="xw", bufs=6)
                        for k in range(2):
                            pt = pp.tile([P, P], F32R, tag="mask", bufs=2, name="pt")
                            nc.tensor.transpose(
                                pt[:], xchunk[:, k * P : (k + 1) * P], ident_r[:]
                            )
                            nc.vector.tensor_copy(xt[:, k], pt[:])
                        # Wh rows (natural) for this node chunk
                        pa = pp.tile([P, HID], F32, tag="agg", bufs=2, name="pa")
                        for k in range(2):
                            nc.tensor.matmul(
                                pa[:],
                                xt[:, k],
                                Ws_sb[:, k, :],
                                start=(k == 0),
                                stop=(k == 1),
                            )
                        nc.vector.tensor_copy(Wh_aug[:, o, :HID], pa[:])
                        # s for this chunk: X @ (W_s @ [r1 r2]); col 1 = s_j
                        psn = pp.tile([P, 2], F32, tag="bcast", bufs=1, name="psn")
                        for k in range(2):
                            nc.tensor.matmul(
                                psn[:],
                                xt[:, k],
                                w12[:, k],
                                start=(k == 0),
                                stop=(k == 1),
                            )
                        nc.vector.tensor_copy(s_nat[:, o : o + 1], psn[:, 1:2])

                    # local Wh.T (from X_loc) for the s_i row
                    WhlT = s1pool.tile([P, 2, LOC], F32R)
                    for ic in range(LB):
                        xlc = wk.tile([P, HID], F32R, tag="xw", bufs=6)
                        nc.scalar.dma_start(
                            xlc[:],
                            Xloc_d.ap()[ic * P : (ic + 1) * P, :].bitcast(F32R),
                        )
                        xlt = wk.tile([P, 2, P], F32R, tag="xw", bufs=6)
                        for k in range(2):
                            pt2 = pp.tile([P, P], F32R, tag="mask", bufs=2,
                                          name="pt2")
                            nc.tensor.transpose(
                                pt2[:], xlc[:, k * P : (k + 1) * P], ident_r[:]
                            )
                            nc.vector.tensor_copy(xlt[:, k], pt2[:])
                        for m2 in range(2):
                            pw2 = pp.tile([P, P], F32, tag="st", bufs=2, name="pw2")
                            for k in range(2):
                                nc.tensor.matmul(
                                    pw2[:],
                                    Ws_sb[:, k, m2 * P : (m2 + 1) * P],
                                    xlt[:, k],
                                    start=(k == 0),
                                    stop=(k == 1),
                                )
                            nc.vector.tensor_copy(
                                WhlT[:, m2, ic * P : (ic + 1) * P], pw2[:]
                            )

                    psr = pp.tile([2, LOC], F32, tag="aggz", bufs=1, name="psr")
                    for k in range(2):
                        nc.tensor.matmul(
                            psr[:],
                            rp[:, k, :],
                            WhlT[:, k, :],
                            start=(k == 0),
                            stop=(k == 1),
                        )
                    sir = s1pool.tile([1, LOC], F32)
                    nc.vector.tensor_copy(sir[:], psr[0:1, :])
                    B_sb = s1pool.tile([P, LOC], F32)
                    nc.gpsimd.partition_broadcast(B_sb[:], sir[:])

                    # =========== phase 3: stage-1 attention ===========
                    u0 = pp.tile([P, LOC], F32, tag="agg", bufs=2, name="u0")
                    u1 = pp.tile([P, LOC], F32, tag="agg", bufs=2, name="u1")
                    uz = pp.tile([2, LOC], F32, tag="aggz", bufs=1, name="uz")
                    for jc in range(NB):
                        # leaky_relu(s_i + s_j) = max(t, ALPHA*t) on DVE
                        t1 = wk.tile([P, LOC], F32, tag="s1", bufs=6)
                        nc.vector.tensor_scalar(
                            t1[:], B_sb[:], s_nat[:, jc : jc + 1], None,
                            mybir.AluOpType.add,
                        )
                        t2 = wk.tile([P, LOC], F32, tag="s1", bufs=6)
                        nc.vector.tensor_scalar(
                            t2[:], B_sb[:], s_nat[:, jc : jc + 1], ALPHA,
                            mybir.AluOpType.add, mybir.AluOpType.mult,
                        )
                        ex = wk.tile([P, LOC], F32, tag="s1", bufs=6)
                        nc.vector.tensor_max(out=ex[:], in0=t1[:], in1=t2[:])
                        ee = wk.tile([P, LOC], F32, tag="s1", bufs=6)
                        nc.scalar.activation(
                            ee[:], ex[:], mybir.ActivationFunctionType.Exp
                        )
                        em = wk.tile([P, LOC], F32R, tag="s1", bufs=6)
                        nc.vector.tensor_mul(out=em[:], in0=ee[:], in1=At_bf[:, jc])
                        last = jc == NB - 1
                        nc.tensor.matmul(
                            u0[:], Wh_aug[:, jc, 0:P], em[:],
                            start=(jc == 0), stop=last,
                        )
                        nc.tensor.matmul(
                            u1[:], Wh_aug[:, jc, P : 2 * P], em[:],
                            start=(jc == 0), stop=last,
                        )
                        nc.tensor.matmul(
                            uz[:], Wh_aug[:, jc, HID : HID + 2], em[:],
                            start=(jc == 0), stop=last,
                        )

                    # normalize + gelu -> h_local.T [256, 512]
                    zr = s1pool.tile([1, LOC], F32)
                    nc.vector.reciprocal(zr[:], uz[0:1, :])
                    zb = s1pool.tile([P, LOC], F32)
                    nc.gpsimd.partition_broadcast(zb[:], zr[:])
                    for mt, um in enumerate((u0, u1)):
                        tnorm = wk.tile([P, LOC], F32, tag="nrm", bufs=3)
                        nc.vector.tensor_mul(out=tnorm[:], in0=um[:], in1=zb[:])
                        nc.scalar.activation(
                            hT[:, mt], tnorm[:], mybir.ActivationFunctionType.Gelu
                        )

            # =========== phase 4: gathers of h_aug (bf16) and WaT blocks =====
            nc.vector.memset(hnat[:, :, HID : HID + 1], 1.0)
            nc.vector.memset(hnat[:, :, HID + 1 : HID + 2], 0.0)
            for ic in range(LB):
                for fc in range(2):
                    pht = pp.tile([P, P], F32R, tag="st", bufs=2, name="pht")
                    nc.tensor.transpose(
                        pht[:], hT[:, fc, ic * P : (ic + 1) * P], ident_r[:]
                    )
                    nc.vector.tensor_copy(hnat[:, ic, fc * P : (fc + 1) * P],
                                          pht[:])
            nc.scalar.dma_start(
                haug_loc.ap().rearrange("(c p) f -> p c f", p=P), hnat[:]
            )
            # local Wa.T block = W_l.T @ h_local.T
            for m2 in range(2):
                pwa = pp.tile([P, LOC], F32, tag="st", bufs=2, name="pwa")
                for f in range(2):
                    nc.tensor.matmul(
                        pwa[:],
                        Wl_sb[:, f, m2 * P : (m2 + 1) * P],
                        hT[:, f, :],
                        start=(f == 0),
                        stop=(f == 1),
                    )
                nc.vector.tensor_copy(WaTloc[:, m2], pwa[:])
            nc.scalar.dma_start(
                wat_loc.ap().rearrange("(c p) n -> p c n", p=P).bitcast(F32R),
                WaTloc[:],
            )
            nc.gpsimd.collective_compute(
                "AllGather",
                mybir.AluOpType.bypass,
                ins=[haug_loc[:, :]],
                outs=[haug_all[:, :]],
                replica_groups=groups,
            )
            nc.gpsimd.collective_compute(
                "AllGather",
                mybir.AluOpType.bypass,
                ins=[wat_loc[:, :]],
                outs=[wat_all[:, :]],
                replica_groups=groups,
            )

            # =========== mask matmul helper ===========
            def mask_matmul(rhs_tile, out_tile):
                a8_r = a8_all.ap()
                for mg in range(16):
                    pms = [
                        pp.tile([P, LOC], F32, tag="mask", bufs=2, name="pm0"),
                        pp.tile([P, LOC], F32, tag="st", bufs=2, name="pm1"),
                    ]
                    if MASK_MODE == "fp8dr":
                        for kq in range(4):
                            a8t = wk.tile([P, 8, 2 * P], FP8, tag="a8t", bufs=3)
                            src = a8_r.rearrange(
                                "(kq ko p) n -> p ko kq n", p=P, ko=8
                            )
                            nc.sync.dma_start(
                                a8t[:],
                                src[:, :, kq, 2 * P * mg : 2 * P * (mg + 1)],
                            )
                            for s in range(4):
                                for mi in range(2):
                                    nc.tensor.matmul(
                                        pms[mi][:],
                                        a8t[:, 2 * s : 2 * s + 2,
                                            mi * P : (mi + 1) * P],
                                        rhs_tile[:, 8 * kq + 2 * s :
                                                 8 * kq + 2 * s + 2, :],
                                        start=(kq == 0 and s == 0),
                                        stop=(kq == 3 and s == 3),
                                        perf_mode=mybir.MatmulPerfMode.DoubleRow,
                                    )
                    else:
                        for kc in range(NB):
                            a8t2 = wk.tile([P, 2 * P], FP8, tag="a8t", bufs=8)
                            src = a8_r.rearrange("(kc p) n -> p kc n", p=P)
                            nc.sync.dma_start(
                                a8t2[:],
                                src[:, kc, 2 * P * mg : 2 * P * (mg + 1)],
                            )
                            for mi in range(2):
                                nc.tensor.matmul(
                                    pms[mi][:],
                                    a8t2[:, mi * P : (mi + 1) * P],
                                    rhs_tile[:, kc, :],
                                    start=(kc == 0),
                                    stop=(kc == NB - 1),
                                )
                    for mi in range(2):
                        nc.vector.tensor_scalar(
                            out_tile[:, 2 * mg + mi],
                            pms[mi][:],
                            0.5,
                            None,
                            mybir.AluOpType.is_gt,
                        )

            with tc.tile_pool(name="hpool", bufs=1) as hp:
                h_aug = hp.tile([P, NB, HID + 2], BF16, name="h_aug")
                nc.scalar.dma_start(
                    h_aug[:], haug_all.ap().rearrange("(o p) f -> p o f", p=P)
                )
                expS = hp.tile([P, NB, LOC], BF16, name="expS")

                # ---- A^2 mask (PE fills the gather windows) ----
                mask_matmul(M0, M1)

                # ---- scores + expS (needs WaT gather) ----
                with tc.tile_pool(name="scpool", bufs=1) as scpool:
                    WaTall = scpool.tile([P, 2 * NCORES, LOC], F32R)
                    nc.scalar.dma_start(
                        WaTall[:],
                        wat_all.ap().rearrange("(o p) n -> p o n", p=P)
                        .bitcast(F32R),
                    )
                    for m in range(NB):
                        pst = pp.tile([P, LOC], F32, tag="st", bufs=2, name="pst")
                        c, mi = divmod(m, LB)
                        for f in range(2):
                            nc.tensor.matmul(
                                pst[:],
                                WaTall[:, 2 * c + f, mi * P : (mi + 1) * P],
                                hT[:, f, :],
                                start=(f == 0),
                                stop=(f == 1),
                            )
                        nc.scalar.activation(
                            expS[:, m], pst[:], mybir.ActivationFunctionType.Exp
                        )

                # =========== hops ===========
                def hop(mask_fp8, first):
                    u0h = pp.tile([P, LOC], F32, tag="agg", bufs=2, name="u0h")
                    u1h = pp.tile([P, LOC], F32, tag="agg", bufs=2, name="u1h")
                    uzh = pp.tile([2, LOC], F32, tag="aggz", bufs=1, name="uzh")
                    for m in range(NB):
                        ek = wk.tile([P, LOC], BF16, tag="ek", bufs=4)
                        nc.vector.tensor_mul(
                            out=ek[:], in0=expS[:, m], in1=mask_fp8[:, m]
                        )
                        last = m == NB - 1
                        nc.tensor.matmul(
                            u0h[:], h_aug[:, m, 0:P], ek[:],
                            start=(m == 0), stop=last,
                        )
                        nc.tensor.matmul(
                            u1h[:], h_aug[:, m, P : 2 * P], ek[:],
                            start=(m == 0), stop=last,
                        )
                        nc.tensor.matmul(
                            uzh[:], h_aug[:, m, HID : HID + 2], ek[:],
                            start=(m == 0), stop=last,
                        )
                    zrh = wk.tile([1, LOC], F32, tag="row", bufs=2)
                    nc.vector.reciprocal(zrh[:], uzh[0:1, :])
                    zbh = wk.tile([P, LOC], F32, tag="nrm", bufs=3)
                    nc.gpsimd.partition_broadcast(zbh[:], zrh[:])
                    for mt, um in enumerate((u0h, u1h)):
                        tn = wk.tile([P, LOC], F32R, tag="nrm", bufs=3)
                        nc.vector.tensor_mul(out=tn[:], in0=um[:], in1=zbh[:])
                        if first:
                            nc.vector.tensor_add(
                                out=outT[:, mt], in0=hT[:, mt], in1=tn[:]
                            )
                        else:
                            nc.vector.tensor_add(
                                out=outT[:, mt], in0=outT[:, mt], in1=tn[:]
                            )

                hop(M0, first=True)

                # ---- A^3 mask, then remaining hops ----
                mask_matmul(M1, M2)
                hop(M1, first=False)
                hop(M2, first=False)

            # =========== output projection ===========
            py = pp.tile([P, LOC], F32, tag="bcast", bufs=1, name="py")
            for k in range(2):
                nc.tensor.matmul(
                    py[:],
                    Wo_sb[:, k, :],
                    outT[:, k, :],
                    start=(k == 0),
                    stop=(k == 1),
                )
            yt = sm.tile([P, LOC], F32, name="yt")
            nc.vector.tensor_scalar(
                yt[:], py[:], bo_sb[:, 0:1], None, mybir.AluOpType.add
            )
            nc.scalar.dma_start(out_d[:, :], yt[:])

    nc.compile()
    return nc


def _get_nc():
    if "nc" not in _CACHE:
        _CACHE["nc"] = build_kernel()
    return _CACHE["nc"]


def kernel(X, A, W_s, r, W_l, W_out, b_out):
    global last_in_maps
    X = np.ascontiguousarray(X, dtype=np.float32)
    A = np.ascontiguousarray(A, dtype=np.float32)
    in_maps = []
    for c in range(NCORES):
        in_maps.append(
            {
                "X": X,
                "X_loc": np.ascontiguousarray(X[c * LOC : (c + 1) * LOC]),
                "A_blk": np.ascontiguousarray(A[c * LOC : (c + 1) * LOC]),
                "W_s": np.ascontiguousarray(W_s, dtype=np.float32),
                "r": np.ascontiguousarray(r, dtype=np.float32),
                "W_l": np.ascontiguousarray(W_l, dtype=np.float32),
                "W_out": np.ascontiguousarray(W_out, dtype=np.float32),
                "b_out": np.ascontiguousarray(b_out, dtype=np.float32),
            }
        )
    last_in_maps = in_maps
    nc = _get_nc()
    res = run_bass_kernel_spmd(nc, in_maps, core_ids=list(range(NCORES)))
    Y = np.empty((N, OUT_DIM), dtype=np.float32)
    for c in range(NCORES):
        Y[c * LOC : (c + 1) * LOC, :] = res.results[c]["out"].T
    return Y


if __name__ == "__main__":
    build_kernel()
    print("build OK")
